# revision 1
# baseline (speedup 1.0000x reference)
"""Trainium2 Bass kernel for AttentionWithComplexRoPE.

Strategy (8 NeuronCores): data-parallel over batch (B=2) x tensor-parallel
over heads (16 heads -> 4 per core). Core c handles batch c//4, heads
[4*(c%4), 4*(c%4)+4).

Per-core pipeline:
  phase 1: q/k projections in transposed+deinterleaved layout
           qrT/qiT/krT/kiT [128 part = 4 heads x 32 (real|imag) feats, 2048 t]
           + RoPE as aligned elementwise ops; v in [t, f] layout (+ones col).
  phase 2: scores sT[t,s] = k'.T-style matmuls (K=32, 4-way row-packed),
           exp on ScalarE straight from 4-bank PSUM with 1/8 scale folded,
           PV accumulation with ones-column denominator trick (M=65),
           per-head normalize on eviction.
  phase 3: Wo projection per head (K=64 accumulate), partial y to DRAM.
Host: slice/permute weights, transpose x, replicate freqs; sum the 4
per-batch partials at the end.

All matmuls run in float32r (TF32-like) for 4x PE throughput vs fp32.
"""
import sys

if "/opt/trn_rl_repo" not in sys.path:
    sys.path.insert(0, "/opt/trn_rl_repo")

import numpy as np

import concourse.bass as bass
import concourse.mybir as mybir
import concourse.tile as tile
from concourse import bacc
from concourse.bass_utils import run_bass_kernel_spmd

F32 = mybir.dt.float32
F32R = mybir.dt.float32r

B, S, C = 2, 2048, 1024
H = 16                      # global heads
HL = 4                      # heads per core
D = C // H                  # 64
DH = 32                     # complex pairs per head
F = HL * D                  # 256 local features
N_CORES = 8
KT = C // 128               # 8 contraction tiles for projections
TT = S // 128               # 16 token tiles
SC = S // 512               # 4 s-chunks
CC = C // 512               # 2 c-chunks for Wo
SCALE = float(D) ** -0.5

_CACHED_NC = None


def build_module():
    nc = bacc.Bacc("TRN2", target_bir_lowering=False)

    xt = nc.dram_tensor("xt", [C, S], F32R, kind="ExternalInput")
    wqr = nc.dram_tensor("wqr", [128, KT * 128], F32R, kind="ExternalInput")
    wqi = nc.dram_tensor("wqi", [128, KT * 128], F32R, kind="ExternalInput")
    wkr = nc.dram_tensor("wkr", [128, KT * 128], F32R, kind="ExternalInput")
    wki = nc.dram_tensor("wki", [128, KT * 128], F32R, kind="ExternalInput")
    wv = nc.dram_tensor("wv", [128, KT * 256], F32R, kind="ExternalInput")
    wo = nc.dram_tensor("wo", [HL, 64, C], F32R, kind="ExternalInput")
    fr = nc.dram_tensor("fr", [128, S], F32, kind="ExternalInput")
    fi = nc.dram_tensor("fi", [128, S], F32, kind="ExternalInput")
    out = nc.dram_tensor("out", [S, C], F32, kind="ExternalOutput")

    with tile.TileContext(nc) as tc:
        with tc.tile_pool(name="persist", bufs=1) as persist:
            # persistent sbuf tensors
            qr_sb = persist.tile([128, S], F32R)             # rope'd q (reals)
            qi_sb = persist.tile([128, S], F32R)
            kr_sb = persist.tile([128, S], F32R)
            ki_sb = persist.tile([128, S], F32R)
            vaug_sb = persist.tile([128, TT, HL * 65], F32R)  # v + ones col
            att_sb = [persist.tile([64, S], F32R, name=f"att{h}_sb")
                      for h in range(HL)]
            wo_sb = [persist.tile([64, C], F32R, name=f"wo{h}_sb")
                     for h in range(HL)]

            for h in range(HL):
                nc.sync.dma_start(out=wo_sb[h], in_=wo.ap()[h])
            # ones columns of v_aug (col 64 of each head block), all at once
            nc.vector.memset(
                vaug_sb.rearrange("p tt (h d) -> p tt h d", h=HL)
                [:, :, :, 64:65].bitcast(F32), 1.0)

            # ---------------- phase 1: projections + rope ----------------
            xt_r = xt.rearrange("(kt p) s -> p kt s", p=128)
            with tc.tile_pool(name="ph1", bufs=2) as ph1, \
                 tc.tile_pool(name="ph1ps", bufs=4, space="PSUM") as ph1ps, \
                 tc.tile_pool(name="ropet", bufs=2) as ropet:
                w_sb = {}
                for nm, dram in (("kr", wkr), ("ki", wki),
                                 ("qr", wqr), ("qi", wqi)):
                    w = ph1.tile([128, KT * 128], F32R, name=f"w_{nm}",
                                 tag=nm, bufs=1)
                    nc.sync.dma_start(out=w, in_=dram.ap())
                    w_sb[nm] = w
                wv_sb = ph1.tile([128, KT * 256], F32R, tag="wv", bufs=1)
                nc.sync.dma_start(out=wv_sb, in_=wv.ap())
                fr_sb = ph1.tile([128, S], F32, tag="fr", bufs=1)
                fi_sb = ph1.tile([128, S], F32, tag="fi", bufs=1)
                nc.sync.dma_start(out=fr_sb, in_=fr.ap())
                nc.sync.dma_start(out=fi_sb, in_=fi.ap())

                # stream x in 512-token quarters
                for s0 in range(SC):
                    sl = slice(512 * s0, 512 * (s0 + 1))
                    xtq = ph1.tile([128, KT, 512], F32R, tag="xtq", bufs=2)
                    nc.sync.dma_start(out=xtq, in_=xt_r[:, :, sl])

                    # k and q projections + rope for this token chunk
                    for nm, dst_r, dst_i in (("k", kr_sb, ki_sb),
                                             ("q", qr_sb, qi_sb)):
                        wr_, wi_ = w_sb[nm + "r"], w_sb[nm + "i"]
                        ps_r = ph1ps.tile([128, 512], F32, tag="proj")
                        ps_i = ph1ps.tile([128, 512], F32, tag="proj")
                        for kt in range(KT):
                            nc.tensor.matmul(
                                ps_r, wr_[:, 128 * kt:128 * (kt + 1)],
                                xtq[:, kt, :],
                                start=(kt == 0), stop=(kt == KT - 1))
                        for kt in range(KT):
                            nc.tensor.matmul(
                                ps_i, wi_[:, 128 * kt:128 * (kt + 1)],
                                xtq[:, kt, :],
                                start=(kt == 0), stop=(kt == KT - 1))
                        # rope: r' = r*fr - i*fi ; i' = r*fi + i*fr
                        t_rr = ropet.tile([128, 512], F32, tag="t0")
                        t_ii = ropet.tile([128, 512], F32, tag="t1")
                        t_ri = ropet.tile([128, 512], F32, tag="t2")
                        t_ir = ropet.tile([128, 512], F32, tag="t3")
                        nc.vector.tensor_tensor(t_rr, ps_r, fr_sb[:, sl],
                                                op=mybir.AluOpType.mult)
                        nc.vector.tensor_tensor(t_ii, ps_i, fi_sb[:, sl],
                                                op=mybir.AluOpType.mult)
                        nc.vector.tensor_tensor(t_ri, ps_r, fi_sb[:, sl],
                                                op=mybir.AluOpType.mult)
                        nc.vector.tensor_tensor(t_ir, ps_i, fr_sb[:, sl],
                                                op=mybir.AluOpType.mult)
                        nc.vector.tensor_tensor(dst_r[:, sl], t_rr, t_ii,
                                                op=mybir.AluOpType.subtract)
                        nc.vector.tensor_tensor(dst_i[:, sl], t_ri, t_ir,
                                                op=mybir.AluOpType.add)

                    # v projection into [t, f] with ones cols interleaved
                    for tl in range(4):
                        tt = 4 * s0 + tl
                        ps_v = ph1ps.tile([128, 256], F32, tag="proj")
                        for kt in range(KT):
                            nc.tensor.matmul(
                                ps_v, xtq[:, kt, 128 * tl:128 * (tl + 1)],
                                wv_sb[:, 256 * kt:256 * (kt + 1)],
                                start=(kt == 0), stop=(kt == KT - 1))
                        # strided evict: head h -> cols [65h, 65h+64)
                        vv = vaug_sb[:, tt, :].rearrange(
                            "p (h d) -> p h d", h=HL)
                        nc.vector.tensor_copy(
                            vv[:, :, 0:64],
                            ps_v.rearrange("p (h d) -> p h d", h=HL))

            # ---------------- phase 2: attention ----------------
            with tc.tile_pool(name="ph2", bufs=3) as ph2, \
                 tc.tile_pool(name="qkps", bufs=1, space="PSUM") as qkps, \
                 tc.tile_pool(name="accps", bufs=4, space="PSUM") as accps, \
                 tc.tile_pool(name="ph2s", bufs=4) as ph2s:
                for s0 in range(SC):
                    ssl = slice(512 * s0, 512 * (s0 + 1))
                    accs = [accps.tile([65, 512], F32, tag="acc",
                                       name=f"acc{h}") for h in range(HL)]
                    for tt in range(TT):
                        tsl = slice(128 * tt, 128 * (tt + 1))
                        qk = qkps.tile([128, HL * 512], F32, tag="qk")
                        for h in range(HL):
                            hp = slice(32 * h, 32 * (h + 1))
                            osl = slice(512 * h, 512 * (h + 1))
                            nc.tensor.matmul(
                                qk[:, osl], kr_sb[hp, tsl], qr_sb[hp, ssl],
                                start=True, stop=False,
                                tile_position=(32 * h, 0))
                        for h in range(HL):
                            hp = slice(32 * h, 32 * (h + 1))
                            osl = slice(512 * h, 512 * (h + 1))
                            nc.tensor.matmul(
                                qk[:, osl], ki_sb[hp, tsl], qi_sb[hp, ssl],
                                start=False, stop=True,
                                tile_position=(32 * h, 0))
                        exps = ph2.tile([128, HL * 512], F32R, tag="exps")
                        nc.scalar.activation(exps, qk,
                                             mybir.ActivationFunctionType.Exp,
                                             scale=SCALE)
                        for h in range(HL):
                            nc.tensor.matmul(
                                accs[h],
                                vaug_sb[:, tt, 65 * h:65 * (h + 1)],
                                exps[:, 512 * h:512 * (h + 1)],
                                start=(tt == 0), stop=(tt == TT - 1))
                    # normalize: att[h][:, ssl] = acc[0:64] * (1/acc[64])
                    for h in range(HL):
                        recip = ph2s.tile([1, 512], F32, tag="recip")
                        nc.vector.reciprocal(recip, accs[h][64:65, :])
                        bcast = ph2s.tile([64, 512], F32, tag="bcast")
                        nc.gpsimd.partition_broadcast(bcast, recip)
                        nc.vector.tensor_tensor(att_sb[h][:, ssl],
                                                accs[h][0:64, :], bcast,
                                                op=mybir.AluOpType.mult)

            # ---------------- phase 3: output projection ----------------
            with tc.tile_pool(name="ph3", bufs=4) as ph3, \
                 tc.tile_pool(name="ph3ps", bufs=4, space="PSUM") as ph3ps:
                for st in range(TT):
                    tsl = slice(128 * st, 128 * (st + 1))
                    for cc in range(CC):
                        csl = slice(512 * cc, 512 * (cc + 1))
                        ps_y = ph3ps.tile([128, 512], F32, tag="y")
                        for h in range(HL):
                            nc.tensor.matmul(
                                ps_y, att_sb[h][:, tsl],
                                wo_sb[h][:, csl],
                                start=(h == 0), stop=(h == HL - 1))
                        y_sb = ph3.tile([128, 512], F32, tag="y_sb")
                        nc.scalar.copy(y_sb, ps_y)
                        nc.sync.dma_start(out=out.ap()[tsl, csl], in_=y_sb)

    nc.compile()
    return nc


def _round_f32r(a):
    """Round fp32 to the TF32-like fp32r grid (10 mantissa bits, rne)."""
    x = np.ascontiguousarray(a, dtype=np.float32)
    xi = x.view(np.uint32)
    shift = 13
    bias = ((xi >> shift) & 1) + (1 << (shift - 1)) - 1
    xr = (((xi + bias) >> shift) << shift).astype(np.uint32)
    return xr.view(np.float32)


def make_inputs(x, freqs, Wq, Wk, Wv, Wo, round_host=False):
    """Build the 8 per-core input maps."""
    rnd = _round_f32r if round_host else (
        lambda a: np.ascontiguousarray(a, dtype=np.float32))

    # deinterleave permutations of the 256 local feature rows
    p = np.arange(128)
    real_rows = 64 * (p // 32) + 2 * (p % 32)       # within local 256 block
    imag_rows = real_rows + 1

    frh = np.ascontiguousarray(np.tile(freqs[:, :, 0].T, (HL, 1)),
                               dtype=np.float32)    # [128, S]
    fih = np.ascontiguousarray(np.tile(freqs[:, :, 1].T, (HL, 1)),
                               dtype=np.float32)

    def proj_weight(W, rows):
        # lhsT tiles: [128 c-part, KT*128], w[p, kt*128+m] = W[base+rows[m], kt*128+p]
        wt = W[rows, :]                              # [128, C]
        return rnd(wt.T.reshape(KT, 128, 128).transpose(1, 0, 2)
                   .reshape(128, KT * 128))

    in_maps = []
    for c in range(N_CORES):
        b, hg = divmod(c, 4)
        base = 256 * hg
        wqr = proj_weight(Wq, base + real_rows)
        wqi = proj_weight(Wq, base + imag_rows)
        wkr = proj_weight(Wk, base + real_rows)
        wki = proj_weight(Wk, base + imag_rows)
        # v: [128 c-part, KT*256], wv[p, kt*256+f] = Wv[base+f, kt*128+p]
        wvt = Wv[base:base + F, :].T                 # [C, F]
        wv_ = rnd(wvt.reshape(KT, 128, F).transpose(1, 0, 2)
                  .reshape(128, KT * F))
        # wo: [HL, 64, C], head h rows = Wo columns for that head, transposed
        wo_ = np.empty((HL, 64, C), np.float32)
        for h in range(HL):
            wo_[h] = Wo[:, base + 64 * h: base + 64 * (h + 1)].T
        in_maps.append({
            "xt": rnd(x[b].T),
            "wqr": wqr, "wqi": wqi, "wkr": wkr, "wki": wki,
            "wv": wv_, "wo": rnd(wo_),
            "fr": frh, "fi": fih,
        })
    return in_maps


def kernel(x, freqs, Wq, Wk, Wv, Wo):
    global _CACHED_NC
    x = np.asarray(x, dtype=np.float32)
    freqs = np.asarray(freqs, dtype=np.float32)
    Wq = np.asarray(Wq, dtype=np.float32)
    Wk = np.asarray(Wk, dtype=np.float32)
    Wv = np.asarray(Wv, dtype=np.float32)
    Wo = np.asarray(Wo, dtype=np.float32)

    in_maps = make_inputs(x, freqs, Wq, Wk, Wv, Wo)
    if _CACHED_NC is None:
        _CACHED_NC = build_module()
    res = run_bass_kernel_spmd(_CACHED_NC, in_maps,
                               core_ids=list(range(N_CORES)))
    outs = [r["out"] for r in res.results]
    y = np.empty((B, S, C), np.float32)
    for b in range(B):
        y[b] = outs[4 * b] + outs[4 * b + 1] + outs[4 * b + 2] + outs[4 * b + 3]
    return y


if __name__ == "__main__":
    rng = np.random.default_rng(0)
    x = rng.standard_normal((B, S, C)).astype(np.float32)
    freqs = rng.standard_normal((S, DH, 2)).astype(np.float32)
    ws = [(rng.standard_normal((C, C)) * C ** -0.5).astype(np.float32)
          for _ in range(4)]
    y = kernel(x, freqs, *ws)
    print("out", y.shape, y.dtype, float(np.abs(y).mean()))



# revision 18
# speedup vs baseline: 1.3121x; 1.3121x over previous
"""Trainium2 Bass kernel for AttentionWithComplexRoPE.

Strategy (8 NeuronCores): data-parallel over batch (B=2) x tensor-parallel
over heads (16 heads -> 4 per core). Core c handles batch c//4, heads
[4*(c%4), 4*(c%4)+4).

Cost-model facts driving the design: a matmul instruction costs
N_out cycles regardless of K and M (fp32r at N>=256 runs 1 cycle/row),
and exp runs only on the Activation engine at 1 elem/lane/cycle
(1.2 GHz) -> the intrinsic exp work (4 heads x 2048^2 / 128 lanes
~ 109 us) roughly matches the minimal PE stream (~164 us). So:
maximize K per matmul (K=64 scores via stacked real|imag rows, K=128
Wo via stacked head pairs), keep the exp stream dense, and hide all
remaining work (deferred q projections, Wo, evicts) in PE/DVE slack
under it.

Layout: heads grouped in pairs j in {0,1} (heads 2j, 2j+1). q/k tiles
qri[j]/kri[j] are [128, S] with rows = [head 2j: 64 | head 2j+1: 64],
within a head the 64 rows are 16-interleaved: [r0-15, i0-15, r16-31,
i16-31] so that RoPE's r<->i operand swap is a stream_shuffle (which
permutes within 32-row quadrants). RoPE itself is y = ps*F1 + sh*F2
with host-prepared F1 = fr rows, F2 = -/+fi rows (sign per r/i block).

Schedule: phase 1 streams x in 512-token chunks computing k and v for
all chunks but q only for chunk 0. Then 8 attention passes, one per
(query chunk s0, head pair j): per token tile tt, two K=64 score
matmuls into a double-buffered 2-bank PSUM tile, one exp [128,1024] on
the Activation engine, PV (K=128, M=65 incl. ones-column denominator)
delayed one tt. PSUM budget: qk 2x2 + accs 2 + proj 1 + Wo-y 1 = 8
banks, which is what lets the deferred q projection (x re-fetched by
DMA) and the previous chunk's Wo run inside the passes.
Host: permute/slice weights, replicate freqs; sum 4 partials per batch.
"""
import sys

if "/opt/trn_rl_repo" not in sys.path:
    sys.path.insert(0, "/opt/trn_rl_repo")

import ml_dtypes
import numpy as np

import concourse.bass as bass
import concourse.mybir as mybir
import concourse.tile as tile
from concourse import bacc
from concourse.bass_utils import run_bass_kernel_spmd

F32 = mybir.dt.float32
F32R = mybir.dt.float32r
BF16 = mybir.dt.bfloat16

B, S, C = 2, 2048, 1024
H = 16                      # global heads
HL = 4                      # heads per core
NP = 2                      # head pairs per core
D = C // H                  # 64
F = HL * D                  # 256 local features
N_CORES = 8
KT = C // 128               # 8 contraction tiles for projections
TT = S // 128               # 16 token tiles
SC = S // 512               # 4 s-chunks
CC = C // 512               # 2 c-chunks for Wo
SCALE = float(D) ** -0.5
SWAP16 = [(i + 16) % 32 for i in range(32)]   # r<->i within quadrants

_CACHED_NC = None
DEBUG = False


def build_module():
    nc = bacc.Bacc("TRN2", target_bir_lowering=False)

    xt = nc.dram_tensor("xt", [C, S], BF16, kind="ExternalInput")
    wq = [nc.dram_tensor(f"wq{j}", [128, KT * 128], BF16,
                         kind="ExternalInput") for j in range(NP)]
    wk = [nc.dram_tensor(f"wk{j}", [128, KT * 128], BF16,
                         kind="ExternalInput") for j in range(NP)]
    wv = nc.dram_tensor("wv", [128, KT * 256], BF16, kind="ExternalInput")
    wo = [nc.dram_tensor(f"wo{j}", [128, C], F32R,
                         kind="ExternalInput") for j in range(NP)]
    f1 = nc.dram_tensor("f1", [128, S], F32, kind="ExternalInput")
    f2 = nc.dram_tensor("f2", [128, S], F32, kind="ExternalInput")
    out = nc.dram_tensor("out", [S, C], F32, kind="ExternalOutput")
    dbg = {}
    if DEBUG:
        for nm in ("kri0", "kri1", "qri0", "qri1", "att0", "att1"):
            dbg[nm] = nc.dram_tensor(nm, [128, S], F32, kind="ExternalOutput")
        dbg["vaug0"] = nc.dram_tensor("vaug0", [128, TT * 130], F32,
                                      kind="ExternalOutput")

    xt_r = xt.rearrange("(kt p) s -> p kt s", p=128)

    with tile.TileContext(nc) as tc:
        with tc.tile_pool(name="persist", bufs=1) as persist, \
             tc.tile_pool(name="span", bufs=1) as span, \
             tc.tile_pool(name="ropet", bufs=2) as ropet:
            qri = [persist.tile([128, S], F32R, name=f"qri{j}")
                   for j in range(NP)]
            kri = [persist.tile([128, S], F32R, name=f"kri{j}")
                   for j in range(NP)]
            # v + ones col per head: [t-part, tt, (hh, 65)]
            vaug = [persist.tile([128, TT, 2 * 65], F32R, name=f"vaug{j}")
                    for j in range(NP)]
            att = [persist.tile([128, S], F32R, name=f"att{j}")
                   for j in range(NP)]
            wo_sb = [persist.tile([128, C], F32R, name=f"wo{j}_sb")
                     for j in range(NP)]

            # tensors alive through both phases
            wq_sb = [span.tile([128, KT * 128], BF16, name=f"w_q{j}",
                               tag=f"q{j}") for j in range(NP)]
            f1_sb = span.tile([128, S], F32, tag="f1")
            f2_sb = span.tile([128, S], F32, tag="f2")

            def proj_qk(w, xtq, dst, sl, ps_pool, bufs=2):
                """Project one q/k pair tile for token range sl + rope."""
                ps = ps_pool.tile([128, 512], F32, tag="proj", bufs=bufs)
                for kt in range(KT):
                    nc.tensor.matmul(ps, w[:, 128 * kt:128 * (kt + 1)],
                                     xtq[:, kt, :],
                                     start=(kt == 0), stop=(kt == KT - 1))
                sh = ropet.tile([128, 512], F32, tag="sh")
                nc.vector.stream_shuffle(sh, ps, SWAP16)
                nc.vector.tensor_tensor(dst, ps, f1_sb[:, sl],
                                        op=mybir.AluOpType.mult)
                t = ropet.tile([128, 512], F32, tag="t")
                nc.vector.tensor_tensor(t, sh, f2_sb[:, sl],
                                        op=mybir.AluOpType.mult)
                nc.vector.tensor_tensor(dst, dst, t, op=mybir.AluOpType.add)

            # ---------------- phase 1: k, v (all chunks) + q chunk 0 ------
            with tc.tile_pool(name="ph1", bufs=2) as ph1, \
                 tc.tile_pool(name="ph1ps", bufs=1, space="PSUM") as ph1ps:
                # DMA issue order follows need-time; two HWDGE rings
                # (sync, scalar) carry the transfers in parallel.
                wk_sb = {}
                for j in range(NP):
                    w = ph1.tile([128, KT * 128], BF16, name=f"w_k{j}",
                                 tag=f"k{j}", bufs=1)
                    nc.scalar.dma_start(out=w, in_=wk[j].ap())
                    wk_sb[j] = w
                xtqs = []
                for s0 in range(SC):
                    xtqs.append(ph1.tile([128, KT, 512], BF16, tag="xtq",
                                         bufs=2, name=f"xtq{s0}"))
                sl0 = slice(0, 512)
                for kt in range(0, KT, 2):
                    nc.sync.dma_start(out=xtqs[0][:, kt:kt + 2, :],
                                      in_=xt_r[:, kt:kt + 2, sl0])
                wv_sb = ph1.tile([128, KT * 256], BF16, tag="wv", bufs=1)
                nc.scalar.dma_start(out=f2_sb, in_=f2.ap())
                nc.sync.dma_start(out=f1_sb, in_=f1.ap())
                nc.scalar.dma_start(out=wv_sb, in_=wv.ap())
                for j in range(NP):
                    nc.vector.memset(
                        vaug[j].rearrange("p tt (hh c) -> p tt hh c", hh=2)
                        [:, :, :, 64:65].bitcast(F32), 1.0)
                def fetch_chunk(s0):
                    sl = slice(512 * s0, 512 * (s0 + 1))
                    ring = nc.scalar if s0 % 2 else nc.sync
                    for kt in range(0, KT, 4):
                        ring.dma_start(out=xtqs[s0][:, kt:kt + 4, :],
                                       in_=xt_r[:, kt:kt + 4, sl])

                fetch_chunk(1)
                for j in range(NP):
                    nc.sync.dma_start(out=wq_sb[j], in_=wq[j].ap())
                for j in range(NP):
                    nc.scalar.dma_start(out=wo_sb[j], in_=wo[j].ap())

                for s0 in range(SC):
                    sl = slice(512 * s0, 512 * (s0 + 1))
                    xtq = xtqs[s0]
                    if s0 + 2 < SC:
                        fetch_chunk(s0 + 2)

                    for j in range(NP):
                        proj_qk(wk_sb[j], xtq, kri[j][:, sl], sl, ph1ps)
                    # v projection into [t, f] with ones cols interleaved
                    for tl in range(4):
                        tt = 4 * s0 + tl
                        ps_v = ph1ps.tile([128, 256], F32, tag="psv", bufs=2)
                        for kt in range(KT):
                            nc.tensor.matmul(
                                ps_v, xtq[:, kt, 128 * tl:128 * (tl + 1)],
                                wv_sb[:, 256 * kt:256 * (kt + 1)],
                                start=(kt == 0), stop=(kt == KT - 1))
                        for j in range(NP):
                            vv = vaug[j][:, tt, :].rearrange(
                                "p (hh c) -> p hh c", hh=2)
                            pv = ps_v[:, 128 * j:128 * (j + 1)].rearrange(
                                "p (hh c) -> p hh c", hh=2)
                            nc.scalar.copy(vv[:, :, 0:64], pv)
                    if s0 == 0:
                        for j in range(NP):
                            proj_qk(wq_sb[j], xtq, qri[j][:, sl], sl, ph1ps)

            # ------- phase 2: one attention pass per (s-chunk, head pair) --
            with tc.tile_pool(name="ph2", bufs=2) as ph2, \
                 tc.tile_pool(name="pqps", bufs=1, space="PSUM") as pqps, \
                 tc.tile_pool(name="qkps", bufs=1, space="PSUM") as qkps, \
                 tc.tile_pool(name="accps", bufs=1, space="PSUM") as accps, \
                 tc.tile_pool(name="psyps", bufs=1, space="PSUM") as psyps:

                # x re-fetch for the deferred q projections, one pass ahead
                def fetch_xtq2(chunk):
                    t = ph2.tile([128, KT, 512], BF16, tag="xtq2", bufs=2)
                    nc.sync.dma_start(
                        out=t, in_=xt_r[:, :, 512 * chunk:512 * (chunk + 1)])
                    return t

                xtq2_next = fetch_xtq2(1)
                xtq2 = None
                for s0 in range(SC):
                    ssl = slice(512 * s0, 512 * (s0 + 1))
                    for j in range(NP):
                        if j == 0:
                            qsl = slice(512 * (s0 + 1), 512 * (s0 + 2))
                            xtq2, xtq2_next = xtq2_next, None
                        elif s0 + 2 < SC:
                            xtq2_next = fetch_xtq2(s0 + 2)
                        accs = [accps.tile([65, 512], F32, tag=f"acc{hh}",
                                           name=f"acc{s0}_{j}_{hh}")
                                for hh in range(2)]

                        def emit_pv(ex, tt):
                            for hh in range(2):
                                nc.tensor.matmul(
                                    accs[hh],
                                    vaug[j][:, tt, 65 * hh:65 * (hh + 1)],
                                    ex[:, 512 * hh:512 * (hh + 1)],
                                    start=(tt == 0), stop=(tt == TT - 1))

                        # deferred work, spread one small piece per tt so
                        # the exp stream never starves: q projection for
                        # chunk s0+1 over tts 2-9, Wo(s0-1) over tts 10-13
                        ps_q = None
                        wo_ysb = None

                        def deferred(tt):
                            nonlocal ps_q, wo_ysb
                            if 2 <= tt <= 9 and s0 + 1 < SC:
                                kt = tt - 2
                                if kt == 0:
                                    ps_q = pqps.tile([128, 512], F32,
                                                     tag="proj")
                                nc.tensor.matmul(
                                    ps_q,
                                    wq_sb[j][:, 128 * kt:128 * (kt + 1)],
                                    xtq2[:, kt, :],
                                    start=(kt == 0), stop=(kt == KT - 1))
                                if kt == KT - 1:
                                    dst = qri[j][:, qsl]
                                    sh = ropet.tile([128, 512], F32,
                                                    tag="sh")
                                    nc.vector.stream_shuffle(sh, ps_q,
                                                             SWAP16)
                                    nc.vector.tensor_tensor(
                                        dst, ps_q, f1_sb[:, qsl],
                                        op=mybir.AluOpType.mult)
                                    t = ropet.tile([128, 512], F32,
                                                   tag="t")
                                    nc.vector.tensor_tensor(
                                        t, sh, f2_sb[:, qsl],
                                        op=mybir.AluOpType.mult)
                                    nc.gpsimd.tensor_tensor(
                                        dst, dst, t,
                                        op=mybir.AluOpType.add)
                            elif 10 <= tt <= 13 and s0 >= 1:
                                tl, cc = divmod(tt - 10, 2)
                                st = 4 * (s0 - 1) + 2 * j + tl
                                tsl2 = slice(128 * st, 128 * (st + 1))
                                csl = slice(512 * cc, 512 * (cc + 1))
                                if cc == 0:
                                    wo_ysb = ph2.tile([128, C], F32,
                                                      tag="y_sb")
                                ps_y = psyps.tile([128, 512], F32, tag="y")
                                for jj in range(NP):
                                    nc.tensor.matmul(
                                        ps_y, att[jj][:, tsl2],
                                        wo_sb[jj][:, csl],
                                        start=(jj == 0), stop=(jj == NP - 1))
                                nc.vector.tensor_copy(wo_ysb[:, csl], ps_y)
                                if cc == 1:
                                    nc.sync.dma_start(
                                        out=out.ap()[tsl2, :], in_=wo_ysb)

                        prev = None
                        for tt in range(TT):
                            tsl = slice(128 * tt, 128 * (tt + 1))
                            qk = qkps.tile([128, 1024], F32, tag="qk",
                                           bufs=2)
                            for hh in range(2):
                                hp = slice(64 * hh, 64 * (hh + 1))
                                nc.tensor.matmul(
                                    qk[:, 512 * hh:512 * (hh + 1)],
                                    kri[j][hp, tsl], qri[j][hp, ssl],
                                    start=True, stop=True)
                            ex = ph2.tile([128, 1024], F32R, tag="ex",
                                          bufs=3)
                            nc.scalar.activation(
                                ex, qk, mybir.ActivationFunctionType.Exp,
                                scale=SCALE)
                            if prev is not None:
                                emit_pv(prev, tt - 1)
                            prev = ex
                            deferred(tt)
                        emit_pv(prev, TT - 1)

                        # evict raw accumulators to SBUF so the PSUM banks
                        # free fast (next pass's PV needs them ~1.3us in);
                        # normalize off the critical path from the copies.
                        acc_sb = []
                        for hh in range(2):
                            a = ph2.tile([65, 512], F32, tag=f"accsb{hh}",
                                         bufs=1)
                            nc.vector.tensor_copy(a, accs[hh])
                            acc_sb.append(a)
                        # att[j][64hh:+64, ssl] = acc[0:64] / acc[64]
                        for hh in range(2):
                            recip = ropet.tile([1, 512], F32, tag="recip")
                            nc.vector.reciprocal(recip, acc_sb[hh][64:65, :])
                            bcast = ropet.tile([64, 512], F32, tag="bcast")
                            nc.gpsimd.partition_broadcast(bcast, recip)
                            nc.vector.tensor_tensor(
                                att[j][64 * hh:64 * (hh + 1), ssl],
                                acc_sb[hh][0:64, :], bcast,
                                op=mybir.AluOpType.mult)
                # tail Wo for the last chunk: double-buffer via the now-idle
                # qk tiles (each [128,1024] = two one-bank halves)
                for tl in range(4):
                    st = 4 * (SC - 1) + tl
                    tsl = slice(128 * st, 128 * (st + 1))
                    qkt = qkps.tile([128, 1024], F32, tag="qk", bufs=2)
                    y_sb = ph2.tile([128, C], F32, tag="y_sb")
                    for cc in range(CC):
                        csl = slice(512 * cc, 512 * (cc + 1))
                        for j in range(NP):
                            nc.tensor.matmul(
                                qkt[:, csl], att[j][:, tsl], wo_sb[j][:, csl],
                                start=(j == 0), stop=(j == NP - 1))
                    nc.vector.tensor_copy(y_sb, qkt)
                    nc.sync.dma_start(out=out.ap()[tsl, :], in_=y_sb)
                if DEBUG:
                    for nm, t in (("kri0", kri[0]), ("kri1", kri[1]),
                                  ("qri0", qri[0]), ("qri1", qri[1]),
                                  ("att0", att[0]), ("att1", att[1])):
                        nc.sync.dma_start(out=dbg[nm].ap(),
                                          in_=t.bitcast(F32))
                    nc.sync.dma_start(
                        out=dbg["vaug0"].ap(),
                        in_=vaug[0].rearrange("p a b -> p (a b)").bitcast(F32))

    nc.compile()
    return nc


def _pair_feat():
    """Within-pair feature index [0,128) for row m of a pair tile.

    Rows: [head hh=m//64: 16-interleaved (r0-15, i0-15, r16-31, i16-31)],
    feature within head = 2*d + ri with d = 16*(b//32) + b%16, ri=(b//16)%2.
    """
    m = np.arange(128)
    hh = m // 64
    b = m % 64
    d = 16 * (b // 32) + b % 16
    ri = (b // 16) % 2
    return 64 * hh + 2 * d + ri, d, ri


def make_inputs(x, freqs, Wq, Wk, Wv, Wo):
    """Build the 8 per-core input maps."""
    cast = lambda a: np.ascontiguousarray(a, dtype=np.float32)
    bcast_ = lambda a: np.ascontiguousarray(a, dtype=np.float32).astype(
        ml_dtypes.bfloat16)

    feat, d_of_row, ri_of_row = _pair_feat()

    # freq tensors in row layout: F1 = fr, F2 = -/+fi (ri=0 -> -fi)
    fr = freqs[:, :, 0].T    # [32, S]
    fi = freqs[:, :, 1].T
    f1h = cast(fr[d_of_row, :])
    sgn = np.where(ri_of_row == 0, -1.0, 1.0)[:, None]
    f2h = cast(fi[d_of_row, :] * sgn)

    def proj_weight(W, rows):
        # lhsT tiles: [128 c-part, KT*128], w[p, kt*128+m] = W[rows[m], kt*128+p]
        wt = W[rows, :]                          # [128, C]
        return cast(wt.T.reshape(KT, 128, 128).transpose(1, 0, 2)
                    .reshape(128, KT * 128))

    in_maps = []
    for c in range(N_CORES):
        b, hg = divmod(c, 4)
        base = 256 * hg
        im = {"xt": bcast_(x[b].T), "f1": f1h, "f2": f2h}
        for j in range(NP):
            rows = base + 128 * j + feat
            im[f"wq{j}"] = bcast_(proj_weight(Wq, rows))
            im[f"wk{j}"] = bcast_(proj_weight(Wk, rows))
            im[f"wo{j}"] = cast(
                Wo[:, base + 128 * j: base + 128 * (j + 1)].T)
        # v: [128 c-part, KT*256], wv[p, kt*256+f] = Wv[base+f, kt*128+p]
        wvt = Wv[base:base + F, :].T             # [C, F]
        im["wv"] = bcast_(wvt.reshape(KT, 128, F).transpose(1, 0, 2)
                          .reshape(128, KT * F))
        in_maps.append(im)
    return in_maps


def kernel(x, freqs, Wq, Wk, Wv, Wo):
    global _CACHED_NC
    x = np.asarray(x, dtype=np.float32)
    freqs = np.asarray(freqs, dtype=np.float32)
    Wq = np.asarray(Wq, dtype=np.float32)
    Wk = np.asarray(Wk, dtype=np.float32)
    Wv = np.asarray(Wv, dtype=np.float32)
    Wo = np.asarray(Wo, dtype=np.float32)

    in_maps = make_inputs(x, freqs, Wq, Wk, Wv, Wo)
    if _CACHED_NC is None:
        _CACHED_NC = build_module()
    res = run_bass_kernel_spmd(_CACHED_NC, in_maps,
                               core_ids=list(range(N_CORES)))
    outs = [r["out"] for r in res.results]
    y = np.empty((B, S, C), np.float32)
    for b in range(B):
        y[b] = outs[4 * b] + outs[4 * b + 1] + outs[4 * b + 2] + outs[4 * b + 3]
    return y


if __name__ == "__main__":
    rng = np.random.default_rng(0)
    x = rng.standard_normal((B, S, C)).astype(np.float32)
    freqs = rng.standard_normal((S, D // 2, 2)).astype(np.float32)
    ws = [(rng.standard_normal((C, C)) * C ** -0.5).astype(np.float32)
          for _ in range(4)]
    y = kernel(x, freqs, *ws)
    print("out", y.shape, y.dtype, float(np.abs(y).mean()))


# revision 20
# speedup vs baseline: 1.4811x; 1.1288x over previous
"""Trainium2 Bass kernel for AttentionWithComplexRoPE.

Strategy (8 NeuronCores): data-parallel over batch (B=2) x tensor-parallel
over heads (16 heads -> 4 per core). Core c handles batch c//4, heads
[4*(c%4), 4*(c%4)+4).

Cost-model facts driving the design: a matmul instruction costs
N_out cycles regardless of K and M (fp32r at N>=256 runs 1 cycle/row),
and exp runs only on the Activation engine at 1 elem/lane/cycle
(1.2 GHz) -> the intrinsic exp work (4 heads x 2048^2 / 128 lanes
~ 109 us) roughly matches the minimal PE stream (~164 us). So:
maximize K per matmul (K=64 scores via stacked real|imag rows, K=128
Wo via stacked head pairs), keep the exp stream dense, and hide all
remaining work (deferred q projections, Wo, evicts) in PE/DVE slack
under it.

Layout: heads grouped in pairs j in {0,1} (heads 2j, 2j+1). q/k tiles
qri[j]/kri[j] are [128, S] with rows = [head 2j: 64 | head 2j+1: 64],
within a head the 64 rows are 16-interleaved: [r0-15, i0-15, r16-31,
i16-31] so that RoPE's r<->i operand swap is a stream_shuffle (which
permutes within 32-row quadrants). RoPE itself is y = ps*F1 + sh*F2
with host-prepared F1 = fr rows, F2 = -/+fi rows (sign per r/i block).

Schedule: phase 1 streams x in 512-token chunks computing k and v for
all chunks but q only for chunk 0. Then 8 attention passes, one per
(query chunk s0, head pair j): per token tile tt, two K=64 score
matmuls into a double-buffered 2-bank PSUM tile, one exp [128,1024] on
the Activation engine, PV (K=128, M=65 incl. ones-column denominator)
delayed one tt. PSUM budget: qk 2x2 + accs 2 + proj 1 + Wo-y 1 = 8
banks, which is what lets the deferred q projection (x re-fetched by
DMA) and the previous chunk's Wo run inside the passes.
Host: permute/slice weights, replicate freqs; sum 4 partials per batch.
"""
import sys

if "/opt/trn_rl_repo" not in sys.path:
    sys.path.insert(0, "/opt/trn_rl_repo")

import ml_dtypes
import numpy as np

import concourse.bass as bass
import concourse.mybir as mybir
import concourse.tile as tile
from concourse import bacc
from concourse.bass_utils import run_bass_kernel_spmd

F32 = mybir.dt.float32
F32R = mybir.dt.float32r
BF16 = mybir.dt.bfloat16

B, S, C = 2, 2048, 1024
H = 16                      # global heads
HL = 4                      # heads per core
NP = 2                      # head pairs per core
D = C // H                  # 64
F = HL * D                  # 256 local features
N_CORES = 8
KT = C // 128               # 8 contraction tiles for projections
TT = S // 128               # 16 token tiles
SC = S // 512               # 4 s-chunks
CC = C // 512               # 2 c-chunks for Wo
SCALE = float(D) ** -0.5
SWAP16 = [(i + 16) % 32 for i in range(32)]   # r<->i within quadrants

_CACHED_NC = None
DEBUG = False


def build_module():
    nc = bacc.Bacc("TRN2", target_bir_lowering=False)

    xt = nc.dram_tensor("xt", [C, S], BF16, kind="ExternalInput")
    wq = [nc.dram_tensor(f"wq{j}", [128, KT * 128], BF16,
                         kind="ExternalInput") for j in range(NP)]
    wk = [nc.dram_tensor(f"wk{j}", [128, KT * 128], BF16,
                         kind="ExternalInput") for j in range(NP)]
    wv = nc.dram_tensor("wv", [128, KT * 256], BF16, kind="ExternalInput")
    wo = [nc.dram_tensor(f"wo{j}", [128, C], F32R,
                         kind="ExternalInput") for j in range(NP)]
    f1 = nc.dram_tensor("f1", [128, S], F32, kind="ExternalInput")
    f2 = nc.dram_tensor("f2", [128, S], F32, kind="ExternalInput")
    out = nc.dram_tensor("out", [S, C], F32, kind="ExternalOutput")
    dbg = {}
    if DEBUG:
        for nm in ("kri0", "kri1", "qri0", "qri1", "att0", "att1"):
            dbg[nm] = nc.dram_tensor(nm, [128, S], F32, kind="ExternalOutput")
        dbg["vaug0"] = nc.dram_tensor("vaug0", [128, TT * 130], F32,
                                      kind="ExternalOutput")

    xt_r = xt.rearrange("(kt p) s -> p kt s", p=128)

    with tile.TileContext(nc) as tc:
        with tc.tile_pool(name="persist", bufs=1) as persist, \
             tc.tile_pool(name="span", bufs=1) as span, \
             tc.tile_pool(name="ropet", bufs=2) as ropet:
            qri = [persist.tile([128, S], F32R, name=f"qri{j}")
                   for j in range(NP)]
            kri = [persist.tile([128, S], F32R, name=f"kri{j}")
                   for j in range(NP)]
            # v + ones col per head: [t-part, tt, (hh, 65)]
            vaug = [persist.tile([128, TT, 2 * 65], F32R, name=f"vaug{j}")
                    for j in range(NP)]
            att = [persist.tile([128, S], F32R, name=f"att{j}")
                   for j in range(NP)]
            wo_sb = [persist.tile([128, C], F32R, name=f"wo{j}_sb")
                     for j in range(NP)]

            # tensors alive through both phases
            wq_sb = [span.tile([128, KT * 128], BF16, name=f"w_q{j}",
                               tag=f"q{j}") for j in range(NP)]
            f1_sb = span.tile([128, S], F32, tag="f1")
            f2_sb = span.tile([128, S], F32, tag="f2")

            def proj_qk(w, xtq, dst, sl, ps_pool, bufs=2):
                """Project one q/k pair tile for token range sl + rope."""
                ps = ps_pool.tile([128, 512], F32, tag="proj", bufs=bufs)
                for kt in range(KT):
                    nc.tensor.matmul(ps, w[:, 128 * kt:128 * (kt + 1)],
                                     xtq[:, kt, :],
                                     start=(kt == 0), stop=(kt == KT - 1))
                sh = ropet.tile([128, 512], F32, tag="sh")
                nc.vector.stream_shuffle(sh, ps, SWAP16)
                nc.vector.tensor_tensor(dst, ps, f1_sb[:, sl],
                                        op=mybir.AluOpType.mult)
                t = ropet.tile([128, 512], F32, tag="t")
                nc.vector.tensor_tensor(t, sh, f2_sb[:, sl],
                                        op=mybir.AluOpType.mult)
                nc.vector.tensor_tensor(dst, dst, t, op=mybir.AluOpType.add)

            # ---------------- phase 1: k, v (all chunks) + q chunk 0 ------
            with tc.tile_pool(name="ph1", bufs=2) as ph1, \
                 tc.tile_pool(name="ph1ps", bufs=1, space="PSUM") as ph1ps:
                # DMA issue order follows need-time; two HWDGE rings
                # (sync, scalar) carry the transfers in parallel.
                wk_sb = {}
                for j in range(NP):
                    w = ph1.tile([128, KT * 128], BF16, name=f"w_k{j}",
                                 tag=f"k{j}", bufs=1)
                    nc.scalar.dma_start(out=w, in_=wk[j].ap())
                    wk_sb[j] = w
                xtqs = []
                for s0 in range(SC):
                    xtqs.append(ph1.tile([128, KT, 512], BF16, tag="xtq",
                                         bufs=2, name=f"xtq{s0}"))
                sl0 = slice(0, 512)
                for kt in range(0, KT, 2):
                    nc.sync.dma_start(out=xtqs[0][:, kt:kt + 2, :],
                                      in_=xt_r[:, kt:kt + 2, sl0])
                wv_sb = ph1.tile([128, KT * 256], BF16, tag="wv", bufs=1)
                nc.scalar.dma_start(out=f2_sb, in_=f2.ap())
                nc.sync.dma_start(out=f1_sb, in_=f1.ap())
                nc.scalar.dma_start(out=wv_sb, in_=wv.ap())
                for j in range(NP):
                    nc.vector.memset(
                        vaug[j].rearrange("p tt (hh c) -> p tt hh c", hh=2)
                        [:, :, :, 64:65].bitcast(F32), 1.0)
                def fetch_chunk(s0):
                    sl = slice(512 * s0, 512 * (s0 + 1))
                    ring = nc.scalar if s0 % 2 else nc.sync
                    for kt in range(0, KT, 4):
                        ring.dma_start(out=xtqs[s0][:, kt:kt + 4, :],
                                       in_=xt_r[:, kt:kt + 4, sl])

                fetch_chunk(1)
                for j in range(NP):
                    nc.sync.dma_start(out=wq_sb[j], in_=wq[j].ap())
                for j in range(NP):
                    nc.scalar.dma_start(out=wo_sb[j], in_=wo[j].ap())

                for s0 in range(SC):
                    sl = slice(512 * s0, 512 * (s0 + 1))
                    xtq = xtqs[s0]
                    if s0 + 2 < SC:
                        fetch_chunk(s0 + 2)

                    for j in range(NP):
                        proj_qk(wk_sb[j], xtq, kri[j][:, sl], sl, ph1ps)
                    # v projection into [t, f] with ones cols interleaved
                    for tl in range(4):
                        tt = 4 * s0 + tl
                        ps_v = ph1ps.tile([128, 256], F32, tag="psv", bufs=2)
                        for kt in range(KT):
                            nc.tensor.matmul(
                                ps_v, xtq[:, kt, 128 * tl:128 * (tl + 1)],
                                wv_sb[:, 256 * kt:256 * (kt + 1)],
                                start=(kt == 0), stop=(kt == KT - 1))
                        for j in range(NP):
                            vv = vaug[j][:, tt, :].rearrange(
                                "p (hh c) -> p hh c", hh=2)
                            pv = ps_v[:, 128 * j:128 * (j + 1)].rearrange(
                                "p (hh c) -> p hh c", hh=2)
                            nc.scalar.copy(vv[:, :, 0:64], pv)
                    if s0 == 0:
                        for j in range(NP):
                            proj_qk(wq_sb[j], xtq, qri[j][:, sl], sl, ph1ps)

            # ------- phase 2: one attention pass per (s-chunk, head pair) --
            with tc.tile_pool(name="ph2", bufs=2) as ph2, \
                 tc.tile_pool(name="pqps", bufs=1, space="PSUM") as pqps, \
                 tc.tile_pool(name="qkps", bufs=1, space="PSUM") as qkps, \
                 tc.tile_pool(name="accps", bufs=1, space="PSUM") as accps, \
                 tc.tile_pool(name="psyps", bufs=1, space="PSUM") as psyps:

                # x re-fetch for the deferred q projections, one pass ahead
                def fetch_xtq2(chunk):
                    t = ph2.tile([128, KT, 512], BF16, tag="xtq2", bufs=2)
                    nc.sync.dma_start(
                        out=t, in_=xt_r[:, :, 512 * chunk:512 * (chunk + 1)])
                    return t

                xtq2_next = fetch_xtq2(1)
                xtq2 = None
                for s0 in range(SC):
                    ssl = slice(512 * s0, 512 * (s0 + 1))
                    for j in range(NP):
                        if j == 0:
                            qsl = slice(512 * (s0 + 1), 512 * (s0 + 2))
                            xtq2, xtq2_next = xtq2_next, None
                        elif s0 + 2 < SC:
                            xtq2_next = fetch_xtq2(s0 + 2)
                        accs = [accps.tile([65, 512], F32, tag=f"acc{hh}",
                                           name=f"acc{s0}_{j}_{hh}")
                                for hh in range(2)]

                        def emit_pv(ex, tt):
                            for hh in range(2):
                                nc.tensor.matmul(
                                    accs[hh],
                                    vaug[j][:, tt, 65 * hh:65 * (hh + 1)],
                                    ex[:, 512 * hh:512 * (hh + 1)],
                                    start=(tt == 0), stop=(tt == TT - 1))

                        # deferred work, spread one small piece per tt so
                        # the exp stream never starves: q projection for
                        # chunk s0+1 over tts 2-9, Wo(s0-1) over tts 10-13
                        ps_q = None
                        wo_ysb = None

                        def deferred(tt):
                            nonlocal ps_q, wo_ysb
                            if 2 <= tt <= 9 and s0 + 1 < SC:
                                kt = tt - 2
                                if kt == 0:
                                    ps_q = pqps.tile([128, 512], F32,
                                                     tag="proj")
                                nc.tensor.matmul(
                                    ps_q,
                                    wq_sb[j][:, 128 * kt:128 * (kt + 1)],
                                    xtq2[:, kt, :],
                                    start=(kt == 0), stop=(kt == KT - 1))
                                if kt == KT - 1:
                                    dst = qri[j][:, qsl]
                                    sh = ropet.tile([128, 512], F32,
                                                    tag="sh")
                                    nc.vector.stream_shuffle(sh, ps_q,
                                                             SWAP16)
                                    nc.vector.tensor_tensor(
                                        dst, ps_q, f1_sb[:, qsl],
                                        op=mybir.AluOpType.mult)
                                    t = ropet.tile([128, 512], F32,
                                                   tag="t")
                                    nc.vector.tensor_tensor(
                                        t, sh, f2_sb[:, qsl],
                                        op=mybir.AluOpType.mult)
                                    nc.gpsimd.tensor_tensor(
                                        dst, dst, t,
                                        op=mybir.AluOpType.add)
                            elif 10 <= tt <= 13 and s0 >= 1:
                                tl, cc = divmod(tt - 10, 2)
                                st = 4 * (s0 - 1) + 2 * j + tl
                                tsl2 = slice(128 * st, 128 * (st + 1))
                                csl = slice(512 * cc, 512 * (cc + 1))
                                if cc == 0:
                                    wo_ysb = ph2.tile([128, C], F32,
                                                      tag="y_sb")
                                ps_y = psyps.tile([128, 512], F32, tag="y")
                                for jj in range(NP):
                                    nc.tensor.matmul(
                                        ps_y, att[jj][:, tsl2],
                                        wo_sb[jj][:, csl],
                                        start=(jj == 0), stop=(jj == NP - 1))
                                nc.vector.tensor_copy(wo_ysb[:, csl], ps_y)
                                if cc == 1:
                                    nc.sync.dma_start(
                                        out=out.ap()[tsl2, :], in_=wo_ysb)

                        prev = None
                        for tt in range(TT):
                            tsl = slice(128 * tt, 128 * (tt + 1))
                            qk = qkps.tile([128, 1024], F32, tag="qk",
                                           bufs=2)
                            for hh in range(2):
                                hp = slice(64 * hh, 64 * (hh + 1))
                                nc.tensor.matmul(
                                    qk[:, 512 * hh:512 * (hh + 1)],
                                    kri[j][hp, tsl], qri[j][hp, ssl],
                                    start=True, stop=True)
                            ex = ph2.tile([128, 1024], F32R, tag="ex",
                                          bufs=3)
                            nc.scalar.activation(
                                ex, qk, mybir.ActivationFunctionType.Exp,
                                scale=SCALE)
                            if prev is not None:
                                emit_pv(prev, tt - 1)
                            prev = ex
                            deferred(tt)
                        emit_pv(prev, TT - 1)

                        # evict raw accumulators to SBUF so the PSUM banks
                        # free fast (next pass's PV needs them ~1.3us in);
                        # normalize off the critical path from the copies.
                        acc_sb = []
                        for hh in range(2):
                            a = ph2.tile([65, 512], F32, tag=f"accsb{hh}",
                                         bufs=1)
                            nc.vector.tensor_copy(a, accs[hh])
                            acc_sb.append(a)
                        # att[j][64hh:+64, ssl] = acc[0:64] / acc[64]
                        for hh in range(2):
                            recip = ropet.tile([1, 512], F32, tag="recip")
                            nc.vector.reciprocal(recip, acc_sb[hh][64:65, :])
                            bcast = ropet.tile([64, 512], F32, tag="bcast")
                            nc.gpsimd.partition_broadcast(bcast, recip)
                            nc.vector.tensor_tensor(
                                att[j][64 * hh:64 * (hh + 1), ssl],
                                acc_sb[hh][0:64, :], bcast,
                                op=mybir.AluOpType.mult)
                # tail Wo for the last chunk: double-buffer via the now-idle
                # qk tiles (each [128,1024] = two one-bank halves)
                for tl in range(4):
                    st = 4 * (SC - 1) + tl
                    tsl = slice(128 * st, 128 * (st + 1))
                    qkt = qkps.tile([128, 1024], F32, tag="qk", bufs=2)
                    y_sb = ph2.tile([128, C], F32, tag="y_sb")
                    for cc in range(CC):
                        csl = slice(512 * cc, 512 * (cc + 1))
                        for j in range(NP):
                            nc.tensor.matmul(
                                qkt[:, csl], att[j][:, tsl], wo_sb[j][:, csl],
                                start=(j == 0), stop=(j == NP - 1))
                    nc.vector.tensor_copy(y_sb, qkt)
                    nc.sync.dma_start(out=out.ap()[tsl, :], in_=y_sb)
                if DEBUG:
                    for nm, t in (("kri0", kri[0]), ("kri1", kri[1]),
                                  ("qri0", qri[0]), ("qri1", qri[1]),
                                  ("att0", att[0]), ("att1", att[1])):
                        nc.sync.dma_start(out=dbg[nm].ap(),
                                          in_=t.bitcast(F32))
                    nc.sync.dma_start(
                        out=dbg["vaug0"].ap(),
                        in_=vaug[0].rearrange("p a b -> p (a b)").bitcast(F32))

    nc.compile()
    return nc


def _pair_feat():
    """Within-pair feature index [0,128) for row m of a pair tile.

    Rows: [head hh=m//64: 16-interleaved (r0-15, i0-15, r16-31, i16-31)],
    feature within head = 2*d + ri with d = 16*(b//32) + b%16, ri=(b//16)%2.
    """
    m = np.arange(128)
    hh = m // 64
    b = m % 64
    d = 16 * (b // 32) + b % 16
    ri = (b // 16) % 2
    return 64 * hh + 2 * d + ri, d, ri


def make_inputs(x, freqs, Wq, Wk, Wv, Wo):
    """Build the 8 per-core input maps."""
    cast = lambda a: np.ascontiguousarray(a, dtype=np.float32)
    bcast_ = lambda a: np.ascontiguousarray(a, dtype=np.float32).astype(
        ml_dtypes.bfloat16)

    feat, d_of_row, ri_of_row = _pair_feat()

    # freq tensors in row layout: F1 = fr, F2 = -/+fi (ri=0 -> -fi)
    fr = freqs[:, :, 0].T    # [32, S]
    fi = freqs[:, :, 1].T
    f1h = cast(fr[d_of_row, :])
    sgn = np.where(ri_of_row == 0, -1.0, 1.0)[:, None]
    f2h = cast(fi[d_of_row, :] * sgn)

    def proj_weight(W, rows):
        # lhsT tiles: [128 c-part, KT*128], w[p, kt*128+m] = W[rows[m], kt*128+p]
        wt = W[rows, :]                          # [128, C]
        return cast(wt.T.reshape(KT, 128, 128).transpose(1, 0, 2)
                    .reshape(128, KT * 128))

    xts = [bcast_(x[b].T) for b in range(B)]
    wmaps = []
    for hg in range(4):
        base = 256 * hg
        wm = {}
        for j in range(NP):
            rows = base + 128 * j + feat
            wm[f"wq{j}"] = bcast_(proj_weight(Wq, rows))
            wm[f"wk{j}"] = bcast_(proj_weight(Wk, rows))
            wm[f"wo{j}"] = cast(
                Wo[:, base + 128 * j: base + 128 * (j + 1)].T)
        # v: [128 c-part, KT*256], wv[p, kt*256+f] = Wv[base+f, kt*128+p]
        wvt = Wv[base:base + F, :].T             # [C, F]
        wm["wv"] = bcast_(wvt.reshape(KT, 128, F).transpose(1, 0, 2)
                          .reshape(128, KT * F))
        wmaps.append(wm)
    in_maps = []
    for c in range(N_CORES):
        b, hg = divmod(c, 4)
        im = {"xt": xts[b], "f1": f1h, "f2": f2h, **wmaps[hg]}
        in_maps.append(im)
    return in_maps


def kernel(x, freqs, Wq, Wk, Wv, Wo):
    global _CACHED_NC
    x = np.asarray(x, dtype=np.float32)
    freqs = np.asarray(freqs, dtype=np.float32)
    Wq = np.asarray(Wq, dtype=np.float32)
    Wk = np.asarray(Wk, dtype=np.float32)
    Wv = np.asarray(Wv, dtype=np.float32)
    Wo = np.asarray(Wo, dtype=np.float32)

    in_maps = make_inputs(x, freqs, Wq, Wk, Wv, Wo)
    if _CACHED_NC is None:
        _CACHED_NC = build_module()
    res = run_bass_kernel_spmd(_CACHED_NC, in_maps,
                               core_ids=list(range(N_CORES)))
    outs = [r["out"] for r in res.results]
    y = np.empty((B, S, C), np.float32)
    for b in range(B):
        y[b] = outs[4 * b] + outs[4 * b + 1] + outs[4 * b + 2] + outs[4 * b + 3]
    return y


if __name__ == "__main__":
    rng = np.random.default_rng(0)
    x = rng.standard_normal((B, S, C)).astype(np.float32)
    freqs = rng.standard_normal((S, D // 2, 2)).astype(np.float32)
    ws = [(rng.standard_normal((C, C)) * C ** -0.5).astype(np.float32)
          for _ in range(4)]
    y = kernel(x, freqs, *ws)
    print("out", y.shape, y.dtype, float(np.abs(y).mean()))
